# revision 3
# baseline (speedup 1.0000x reference)
"""Trainium2 Bass kernel for CKANConv2d (KAN conv: SiLU base + B-spline path).

Math: for each output pixel p and output channel co:
  out[co,p] = sum_{c,kh,kw} silu(x[c,p+k]) * Wb[co,(c,kh,kw)]
            + sum_{c,kh,kw,g} B_g(x[c,p+k]) * Ws[co,(c,kh,kw),g]
with B_g the order-3 uniform B-spline bases over knots {-2.2 + 0.4j}.

Key identity used on-chip (t = 2.5*x, center c_g = g - 3.5):
  v = |2.5 x - c_g|; m = min(v,2)-2; n = min(v,1)-1
  B_g(x) = (4 n^3 - m^3) / 6
The 1/6 is folded into the spline weights; the bases are computed per
*input* pixel (not per unfolded patch, 9x less work) and the 3x3
convolution is done as an implicit GEMM over 9 shifted windows with
contraction (c,g) packed 128 rows at a time.

Pipelining: every per-body tile rotates between 2 buffers (tag bufs=2),
so in the replicated timing build body i+1's input DMA + elementwise
basis computation overlap body i's tail matmuls and the PE never
starves. Base matmuls are emitted per-group so each group's PE block
depends only on that group's chunk of elementwise output.

Sharding: data-parallel over batch, 1 image per NeuronCore (8 cores).
"""
import numpy as np
import ml_dtypes

B, CIN, H, W = 8, 64, 56, 56
COUT, K = 128, 3
HO = WO = 54
NOUT = HO * WO  # 2916
NTAP = K * K  # 9
NKT = 4  # spline K-tiles per tap: 128 rows = 64c x 2g, 4 tiles cover g=0..7
NGRP = 6  # output row groups of 9 rows each
GROW = 9  # output rows per group
NFREE = GROW * WO  # 486 <= 512 (one PSUM bank)
RS = 11  # first silu/chunk-0 row boundary
RR = 12  # first x-DMA rows (chunk 0 + 1 shifted-silu row)

_CACHE = {}


def _patch_tile_tail_drain():
    """walrus in this env rejects the Tile tail Drain when it carries >1
    sync waits; split them into a chain of single-wait Drains."""
    import concourse.tile as tile
    from concourse.vector_clock import ScopedClock

    if getattr(tile.TileContext, "_drain_patched", False):
        return

    def _patched(self, tick_clock, wait_clock):
        drain_inst = self.nc.sync.drain()
        wait_clock.add_sem_waits(
            drain_inst.ins, ScopedClock({None: tick_clock.global_clock})
        )
        si = drain_inst.ins.sync_info
        waits = list(si.on_wait) if si is not None else []
        if len(waits) > 1:
            si.on_wait = waits[:1]
            handles = {h.num: h for h in self.sems.allocated().values()}
            for w in waits[1:]:
                extra = self.nc.sync.drain()
                extra.wait_op(handles[w.id], w.wait_value, "sem-ge")
        self.nc.all_engine_barrier()
        assert self.sems is not None
        popped = self.nc._tile_sem_poison_stack.pop()
        assert popped is self._sem_poison
        self.nc.clear_and_free_semaphores(list(self.sems.allocated().values()))
        self.nc.all_engine_barrier()

    tile.TileContext._drain_and_barrier = _patched
    tile.TileContext._drain_patched = True


def _split_excess_waits(nc, max_waits=1):
    """This walrus build encodes at most one sync-wait per instruction.
    Move extra waits onto same-engine NoOps inserted just before."""
    import bass_rust
    from concourse import mybir

    for f in nc.m.functions:
        for bb in f.blocks:
            new = []
            for ins in bb.instructions:
                si = ins.sync_info
                if si is not None and len(si.on_wait) > max_waits:
                    waits = list(si.on_wait)
                    for w in waits[: len(waits) - max_waits]:
                        nop = mybir.InstNoOp(
                            name=nc.get_next_instruction_name(), ins=[], outs=[]
                        )
                        nop.engine = ins.engine
                        h = bass_rust.SemaphoreHandle(name=w.ant_name, num=w.id)
                        bass_rust.wait_op(nop, h, w.wait_value, "sem-ge", False)
                        nc.register_instruction(nop, overwrite=True)
                        new.append(nop)
                    si.on_wait = waits[len(waits) - max_waits :]
                new.append(ins)
            bb.instructions = new


CFG = {
    "nchunks": 6,
    "ew16": True,
    "sq_engine": "act",
    "evict_engine": "dve",
    "swdgeq": 4,
    "wdma": "sync",
    "sbufs": 2,
    "pbufs": 6,
    "cbufs": 2,
}


def _chunks_for(nchunks):
    """Partition input rows 0..55 into nchunks contiguous chunks such that
    matmul group r (needs input rows 9r..9r+10) only depends on chunks
    emitted at or before group r. Returns list of (r0, r1, first_group)
    where first_group is the earliest group index that must wait for it."""
    gper = [len(x) for x in np.array_split(np.arange(NGRP), nchunks)]
    out = []
    g0 = 0
    r_prev = 0
    for ng in gper:
        glast = g0 + ng - 1
        r1 = min(9 * glast + 11, H)
        out.append((r_prev, r1, g0))
        r_prev = r1
        g0 += ng
    return out


def _build(cfg=None):
    key = ("nc", tuple(sorted((cfg or CFG).items())))
    if key in _CACHE:
        return _CACHE[key]
    cfg = dict(CFG, **(cfg or {}))
    _patch_tile_tail_drain()
    import concourse.bass as bass
    import concourse.tile as tile
    from concourse import mybir

    f32 = mybir.dt.float32
    bf16 = mybir.dt.bfloat16
    ew = bf16 if cfg["ew16"] else f32
    Alu = mybir.AluOpType
    Act = mybir.ActivationFunctionType

    nc = bass.Bass("TRN2", num_swdge_queues=cfg["swdgeq"])
    x_d = nc.dram_tensor("x", [CIN, H, W], f32, kind="ExternalInput").ap()
    wspl_d = nc.dram_tensor(
        "wspl", [128, NTAP * NKT, 128], bf16, kind="ExternalInput"
    ).ap()
    # base weights arranged in concurrent row-tile pairs: pair j holds tap 2j
    # on partitions 0:64 and tap 2j+1 on 64:128 (tap 8 alone in pair 4).
    wbase_d = nc.dram_tensor("wbase2", [128, 5, 128], bf16, kind="ExternalInput").ap()
    bneg_d = nc.dram_tensor("betaneg", [128, NKT], f32, kind="ExternalInput").ap()
    y_d = nc.dram_tensor("y", [128, HO, WO], f32, kind="ExternalOutput").ap()

    WB = W * NKT  # 224: 4 k-tile column blocks side by side
    CB = cfg["cbufs"]

    nrep = cfg.get("replicate", 1)
    with tile.TileContext(nc) as tc:
        with (
            tc.tile_pool(name="consts", bufs=1) as cpool,
            tc.tile_pool(name="scratch", bufs=cfg["sbufs"]) as spool,
            tc.tile_pool(name="psum", bufs=cfg["pbufs"], space="PSUM") as ppool,
        ):
          for _rep in range(nrep):
              bneg = cpool.tile([128, NKT], f32, tag="bneg", bufs=CB)
              nc.sync.dma_start(bneg[:], bneg_d)
              # x first, row-chunked so chunk-0 elementwise starts immediately
              x2 = cpool.tile([128, H, W], f32, tag="x2", bufs=CB)
              nc.sync.dma_start(x2[0:CIN, 0:RR, :], x_d[:, 0:RR, :])
              nc.sync.dma_start(x2[CIN:128, 0:RR, :], x_d[:, 0:RR, :])
              wdma = nc.gpsimd.dma_start if cfg["wdma"] == "gpsimd" else nc.sync.dma_start
              wbase = cpool.tile([128, 5, 128], bf16, tag="wbase", bufs=CB)
              wdma(wbase[:], wbase_d)
              wspl = cpool.tile([128, NTAP * NKT, 128], bf16, tag="wspl", bufs=CB)
              wdma(wspl[:, 0:NTAP, :], wspl_d[:, 0:NTAP, :])
              nc.sync.dma_start(x2[0:CIN, RR:H, :], x_d[:, RR:H, :])
              nc.sync.dma_start(x2[CIN:128, RR:H, :], x_d[:, RR:H, :])
              wdma(wspl[:, NTAP : NKT * NTAP, :], wspl_d[:, NTAP : NKT * NTAP, :])

              silu2 = cpool.tile([128, H, W], bf16, tag="silu2", bufs=CB)
              siluB = cpool.tile([128, H, W], bf16, tag="siluB", bufs=CB)
              rhsW = cpool.tile([128, H, WB], bf16, tag="rhsW", bufs=CB)

              def emit_silu(r0, r1):
                  # lower = silu(x) on ACT; shifted upper half via SBUF-SBUF
                  # DMA partition-offset copy (DMA engines are idle here)
                  nc.scalar.activation(
                      silu2[0:CIN, r0:r1, :], x2[0:CIN, r0:r1, :], Act.Silu
                  )
                  # S_A upper: shift (0,+1)
                  nc.sync.dma_start(
                      silu2[CIN:128, r0:r1, 0 : W - 1],
                      silu2[0:CIN, r0:r1, 1:W],
                  )

              def emit_siluB(grp):
                  # S_B rows 9g..9g+8: lower = silu(x); upper shifted (+1,-2)
                  q0, q1 = 9 * grp, 9 * grp + GROW
                  nc.sync.dma_start(
                      siluB[CIN:128, q0:q1, 2:W],
                      silu2[0:CIN, q0 + 1 : q1 + 1, 0 : W - 2],
                  )
                  nc.sync.dma_start(
                      siluB[0:CIN, q0:q1, :], silu2[0:CIN, q0:q1, :]
                  )

              def emit_chunk(r0, r1):
                  rows = r1 - r0
                  sl = (slice(None), slice(r0, r1), slice(0, WB))
                  v = spool.tile([128, rows, WB], f32, tag="v")
                  for t in range(NKT):
                      nc.scalar.activation(
                          v[:, :, t * W : (t + 1) * W],
                          x2[:, r0:r1, :],
                          Act.Abs,
                          bias=bneg[:, t : t + 1],
                          scale=2.5,
                      )
                  m = spool.tile([128, rows, WB], ew, tag="m")
                  nc.vector.tensor_scalar(m[:], v[:], 2.0, 2.0, Alu.min, Alu.subtract)
                  n = spool.tile([128, rows, WB], ew, tag="n")
                  nc.vector.tensor_scalar(n[:], v[:], 1.0, 1.0, Alu.min, Alu.subtract)
                  m2 = spool.tile([128, rows, WB], ew, tag="m2")
                  n2q = spool.tile([128, rows, WB], ew, tag="n2q")
                  if cfg["sq_engine"] == "act":
                      nc.scalar.activation(m2[:], m[:], Act.Square)
                      nc.scalar.activation(n2q[:], n[:], Act.Square, scale=2.0)
                  elif cfg["sq_engine"] == "dve":
                      nc.vector.tensor_tensor(m2[:], m[:], m[:], Alu.mult)
                      nc.vector.scalar_tensor_tensor(
                          n2q[:], n[:], 4.0, n[:], Alu.mult, Alu.mult
                      )
                  elif cfg["sq_engine"] == "pool":
                      nc.gpsimd.tensor_tensor(m2[:], m[:], m[:], Alu.mult)
                      nc.gpsimd.scalar_tensor_tensor(
                          n2q[:], n[:], 4.0, n[:], Alu.mult, Alu.mult
                      )
                  else:  # split: one on act, one on pool
                      nc.scalar.activation(m2[:], m[:], Act.Square)
                      nc.gpsimd.scalar_tensor_tensor(
                          n2q[:], n[:], 4.0, n[:], Alu.mult, Alu.mult
                      )
                  m3 = spool.tile([128, rows, WB], ew, tag="m3")
                  nc.vector.tensor_tensor(m3[:], m2[:], m[:], Alu.mult)
                  n3q = spool.tile([128, rows, WB], ew, tag="n3q")
                  nc.vector.tensor_tensor(n3q[:], n2q[:], n[:], Alu.mult)
                  nc.vector.tensor_tensor(rhsW[sl], n3q[:], m3[:], Alu.subtract)

              chunks = _chunks_for(cfg["nchunks"])

              for grp in range(NGRP):
                  for (r0, r1, g0) in chunks:
                      if g0 == grp:
                          emit_silu(r0, r1)
                          emit_chunk(r0, r1)
                  emit_siluB(grp)

                  ps = ppool.tile([128, NFREE], f32, tag="ps", name=f"ps{grp}")
                  # base path: pairs (0,1*),(2,3),(4,5),(6,7) + tap 8 alone;
                  # pair 1 (taps 2,3) is not a (0,+1) shift -> siluB matmul
                  first = True
                  for j in (0, 2, 3):
                      tapA = 2 * j
                      khA, kwA = divmod(tapA, K)
                      rv = silu2[
                          :, 9 * grp + khA : 9 * grp + khA + GROW, kwA : kwA + WO
                      ]
                      nc.tensor.matmul(
                          ps[:], wbase[:, j, :], rv, start=first, stop=False
                      )
                      first = False
                  sv = silu2[0:CIN, 9 * grp + 2 : 9 * grp + 2 + GROW, 2 : 2 + WO]
                  nc.tensor.matmul(
                      ps[:], wbase[0:CIN, 4, :], sv, start=False, stop=False
                  )
                  rv = siluB[:, 9 * grp : 9 * grp + GROW, 2 : 2 + WO]
                  nc.tensor.matmul(
                      ps[:], wbase[:, 1, :], rv, start=False, stop=False
                  )

                  nsp = NTAP * NKT
                  k = 0
                  for t in range(NKT):
                      for tap in range(NTAP):
                          kh, kw = divmod(tap, K)
                          rv = rhsW[
                              :,
                              9 * grp + kh : 9 * grp + kh + GROW,
                              t * W + kw : t * W + kw + WO,
                          ]
                          nc.tensor.matmul(
                              ps[:],
                              wspl[:, t * NTAP + tap, :],
                              rv,
                              start=False,
                              stop=(k == nsp - 1),
                          )
                          k += 1

                  ev = spool.tile([128, NFREE], f32, tag="ev")
                  if cfg["evict_engine"] == "act":
                      nc.scalar.copy(ev[:], ps[:])
                  elif cfg["evict_engine"] == "pool":
                      nc.gpsimd.tensor_copy(ev[:], ps[:])
                  else:
                      nc.vector.tensor_copy(ev[:], ps[:])
                  nc.sync.dma_start(y_d[:, GROW * grp : GROW * (grp + 1), :], ev[:])

    _split_excess_waits(nc)
    _CACHE[key] = nc
    return nc


def _prep_weights(base_weight, spline_weight, spline_scaler):
    """Fold scaler and 1/6 into spline weights; lay out matmul lhsT tiles."""
    sw = (spline_weight * spline_scaler[:, :, None]).astype(np.float32) / 6.0
    # sw: [COUT, 576, 8]; feature index i = c*9 + tap
    sw4 = sw.reshape(COUT, CIN, NTAP, 8)  # [co, c, tap, g]
    # wspl[p, tap*4+t, co] = sw4[co, c, tap, 2t+gh], p = gh*64 + c
    w = np.transpose(sw4, (1, 2, 3, 0))  # [c, tap, g, co]
    w = w.reshape(CIN, NTAP, NKT, 2, COUT)  # g = 2t + gh -> [c, tap, t, gh, co]
    w = np.transpose(w, (3, 2, 0, 1, 4))  # [gh, t, c, tap, co]
    w = w.reshape(2, NKT, CIN, NTAP, COUT)
    w = np.transpose(w, (0, 2, 1, 3, 4))  # [gh, c, t, tap, co]
    wspl = w.reshape(2 * CIN, NKT * NTAP, COUT).astype(ml_dtypes.bfloat16)

    wb = base_weight.reshape(COUT, CIN, NTAP)  # [co, c, tap]
    wb_ct = np.transpose(wb, (1, 2, 0))  # [c, tap, co]
    wbase = np.zeros((128, 5, COUT), np.float32)
    for j in range(5):
        wbase[0:CIN, j, :] = wb_ct[:, 2 * j, :]
        if j < 4:
            wbase[CIN:128, j, :] = wb_ct[:, 2 * j + 1, :]
    wbase = wbase.astype(ml_dtypes.bfloat16)

    gh = np.arange(128) // CIN  # 0 for p<64, 1 otherwise
    t = np.arange(NKT)
    bneg = (3.5 - (2 * t[None, :] + gh[:, None])).astype(np.float32)  # [128, 4]
    return wspl, wbase, bneg


def _in_maps(x, base_weight, spline_weight, spline_scaler):
    wspl, wbase, bneg = _prep_weights(base_weight, spline_weight, spline_scaler)
    return [
        {
            "x": np.ascontiguousarray(x[b]).astype(np.float32),
            "wspl": wspl,
            "wbase2": wbase,
            "betaneg": bneg,
        }
        for b in range(B)
    ]


def kernel(x, base_weight, spline_weight, spline_scaler):
    from concourse.bass_utils import run_bass_kernel_spmd

    nc = _build()
    in_maps = _in_maps(x, base_weight, spline_weight, spline_scaler)
    res = run_bass_kernel_spmd(nc, in_maps, core_ids=list(range(B)))
    out = np.stack([res.results[b]["y"] for b in range(B)])  # [8, 128, 54, 54]
    return out.astype(np.float32)
